# revision 30
# baseline (speedup 1.0000x reference)
"""NetVLAD pooling kernel for Trainium2 (Bass/Tile), 8-core data-parallel.

Reference computation (per batch b):
    scores = conv_w @ x[b]                  # [K, N]
    assign = softmax(scores, axis=K)
    vlad   = x[b] @ assign.T - centers * assign.sum(n)   # [D, K]
    vlad  /= max(||vlad||_2 over D, eps)    # intra-norm per cluster column
    desc   = vlad.reshape(D*K) / max(||.||_2, eps)

Shapes: x [32, 512, 1024] f32, conv_w [64, 512], centers [512, 64],
output desc [32, 32768] f32.  Sharding: data-parallel over batch,
4 batches per core; params replicated.

v3 strategy (everything fp8 e4m3 on the PE, DoubleRow perf mode):
  * x ships once in fp8 natural layout (2 MB/core) with d split as
    d = 4p + cc (partition p holds 4 consecutive d rows).  conv_w and
    centers are laid out to match, so the vlad output tile
    [128p, 4cc, 64k] maps to CONTIGUOUS 1 KB runs of the desc row ->
    full-bandwidth output DMA with no transposes.
  * scores^T [n, k] via DoubleRow matmuls (contraction 256/instr,
    0.5 cyc/col): 512 PE cycles per batch.  Softmax over k is a
    free-dim reduce (exp cannot overflow; max subtraction dropped).
    E in bf16; AN in fp8 (multiply on GPSIMD for early batches).
  * x^T for the vlad contraction: batches 0-1 GENERATE it on chip
    (DoubleRow matmul against a block-diagonal fp8 identity transposes
    two d-chunks per instruction; one [128,1024] PSUM->SBUF drain per
    quarter-batch, alternating DVE/ACT).  Batches 2-3 SHIP x^T from
    DRAM in the staged layout, with those DMAs queued AFTER the last
    xn so they ride the tail of the serialized DMA stream without
    delaying any batch's softmax.
  * vlad [d, k] via DoubleRow (staged x^T stationary, AN moving);
    the centers term folds into the same PSUM accumulation as one
    bf16 matmul per d-chunk against diag(asum) (DVE: ident64 * asum);
    asum rides as a tiny DoubleRow ones-matmul.
  * PSUM discipline: on this stack start_tensor_calc marks the whole
    2 KB bank pending-zero, so each bank gets exactly ONE start=True
    matmul per lifetime; later first-writes rely on pending-zero
    overwrite semantics (asum / ssq / rinv-broadcast regions are
    packed into the v bank).
  * intra-norm without transposes: ACT Square drains V -> Vsq bf16,
    PE ones-column matmuls accumulate ssq as a ROW [1, 64], DVE
    reciprocal, ACT Sqrt(q/64) folds the global 1/8 (64 unit columns),
    a 1-partition ones matmul broadcasts rinv to [128, 64], one DVE
    multiply reads V from PSUM against the drained broadcast -> f32 out.
  * ACT tables: Square/Copy live in BOTH act tables, Exp and Sqrt do
    not; all four Exps complete before the single Sqrt-table switch,
    so exactly two table loads.
  * PE warms up on dummy DoubleRow matmuls during the DMA lead-in so
    the p-state ramp completes before real work arrives.
"""

import numpy as np
import ml_dtypes

import concourse.bass as bass
from concourse import bacc
import concourse.mybir as mybir
import concourse.tile as tile
from concourse.bass_utils import run_bass_kernel_spmd
from concourse.masks import make_identity

B, D, K, N = 32, 512, 64, 1024
NCORES = 8
BC = B // NCORES          # batches per core (4)
DC = D // 128             # d chunks (4)
NB = N // 128             # n chunks (8)
NJ = NB // 2              # double-n-chunks (4)
NGEN = 2                  # batches whose x^T is generated on chip
F32 = mybir.dt.float32
BF16 = mybir.dt.bfloat16
FP8 = mybir.dt.float8e4
NP_FP8 = np.dtype(ml_dtypes.float8_e4m3)
NP_BF16 = np.dtype(ml_dtypes.bfloat16)
AF = mybir.ActivationFunctionType
DR = mybir.MatmulPerfMode.DoubleRow


def _netvlad_core(ctx, tc, out, xn, xt, w, c):
    """Emit the per-core tile program.

    out: desc [BC, D*K] f32 DRAM
    xn:  x [BC, 128, DC, N] fp8            (xn[b,p,cc,n] = x[b, 4p+cc, n])
    xt:  staged x^T for batches NGEN..BC-1
         [BC-NGEN, NJ, 128, 2, 2, 2, 128] fp8
         (xt[s,J,np,tp,P,u,i] = x[NGEN+s, 4i+2P+u, 256J+128tp+np])
    w:   conv_w^T  [128, DC, K] fp8        (w[p,cc,k] = conv_w[k, 4p+cc])
    c:   NEGATED centers^T [K, DC, 128] bf16 (c[k,cc,i] = -centers[4i+cc, k])
    """
    nc = tc.nc

    const = ctx.enter_context(tc.tile_pool(name="const", bufs=1))
    xpool = ctx.enter_context(tc.tile_pool(name="xp", bufs=1))
    epool = ctx.enter_context(tc.tile_pool(name="ep", bufs=2))
    apool = ctx.enter_context(tc.tile_pool(name="ap", bufs=BC))
    spool = ctx.enter_context(tc.tile_pool(name="sp", bufs=2))
    stpool = ctx.enter_context(tc.tile_pool(name="st", bufs=2))
    vpool = ctx.enter_context(tc.tile_pool(name="vp", bufs=2))
    opool = ctx.enter_context(tc.tile_pool(name="op", bufs=BC))
    # PSUM: s(2x1) + G(4x1) + v(2x1, smalls packed into the v bank) = 8 banks
    ps_s = ctx.enter_context(tc.tile_pool(name="pss", bufs=2, space="PSUM"))
    ps_g = ctx.enter_context(tc.tile_pool(name="psg", bufs=4, space="PSUM"))
    ps_v = ctx.enter_context(tc.tile_pool(name="psv", bufs=2, space="PSUM"))

    # ---- params + constants ------------------------------------------
    wT = const.tile([128, DC, K], FP8, tag="wT")
    nc.sync.dma_start(wT, w)
    cT = const.tile([K, DC, 128], BF16, tag="cT")

    ibig = const.tile([128, 2, 256], FP8, tag="ibig")
    nc.gpsimd.memset(ibig, 0.0)
    make_identity(nc, ibig[:, 0, 0:128], nomemset=True)
    make_identity(nc, ibig[:, 1, 128:256], nomemset=True)
    id64 = const.tile([64, 64], BF16, tag="id64")
    make_identity(nc, id64)
    ones_col = const.tile([128, 1], BF16, tag="ones_col")
    nc.vector.memset(ones_col, 1.0)
    ones_row = const.tile([1, 128], BF16, tag="ones_row")
    nc.vector.memset(ones_row, 1.0)
    ones2 = const.tile([128, 2, 2], FP8, tag="ones2")
    nc.vector.memset(ones2, 1.0)
    # touch Exp immediately so the 1.3us act-table load overlaps the DMAs
    warm = const.tile([1, 2], F32, tag="warm")
    nc.scalar.activation(warm, ones_row[0:1, 0:2], func=AF.Exp)

    # ---- x loads ------------------------------------------------------
    # Serialized-DMA-stream order: params, xn0 (halves), xn1, xn2, xn3
    # (quarters, tail chase), then shipped x^T (b2 first: its AN is
    # ready long before b3's).  14 DMAs keeps the SP sequencer (~0.65us
    # per DMA) at parity with the transfer stream.
    xns = []
    for b in range(BC):
        xtile = xpool.tile([128, 4, DC, N // 4], FP8, tag="xn",
                           name=f"xn{b}", bufs=BC)
        xns.append(xtile)
    stages = {}

    def ship(b):
        sh = stpool.tile([128, NJ, 2, 2, 2, 128], FP8, tag=f"xq{b}",
                         name=f"xq{b}", bufs=1)
        nc.sync.dma_start(
            sh, xt[b - NGEN].rearrange("j p a b c i -> p j a b c i"))
        for J in range(NJ):
            for P in range(2):
                stages[(b, J, P)] = sh[:, J, :, P, :, :]

    def xload(b, nch):
        w_ = 4 // nch
        for h in range(nch):
            nc.sync.dma_start(
                xns[b][:, h * w_:(h + 1) * w_],
                xn[b, h * w_:(h + 1) * w_].rearrange("q p c n -> p q c n"),
            )

    xload(0, 4)
    xload(1, 1)
    nc.sync.dma_start(cT, c)
    xload(2, 1)
    xload(3, 4)
    ship(2)
    ship(3)

    desc_v = out.rearrange("b (p cc k) -> p b cc k", p=128, cc=DC, k=K)

    # ---- PE warmup: dummy DoubleRow matmuls during the DMA lead-in ----
    for i in range(16):
        gd = ps_g.tile([128, 2, 256], F32, tag="g", name=f"warmmm{i}")
        nc.tensor.matmul(gd[:, 0, :], lhsT=ibig[:, :, 0:128],
                         rhs=ibig, start=True, stop=True, perf_mode=DR)

    # ---- per-batch pieces --------------------------------------------
    s_tiles, Es, ANs, reds, recs = {}, {}, {}, {}, {}
    v_tiles = {}
    vsqs = {}
    q_all = spool.tile([1, BC, K], F32, tag="q", bufs=1)
    rinv_all = spool.tile([1, BC, K], BF16, tag="rinv", bufs=1)

    def xsl(b, t, nj):
        q, o = nj // 2, (nj % 2) * 128
        return xns[b][:, q, 2 * t:2 * t + 2, o:o + 128]

    def scores_mms(b, njs):
        # ONE start=True per PSUM bank per batch (see module docstring).
        if b not in s_tiles:
            s_tiles[b] = (ps_s.tile([128, NB, K], F32, tag="s", name=f"s{b}"),
                          [True])
        s_ps, fresh = s_tiles[b]
        for nj in njs:
            for t in range(2):
                nc.tensor.matmul(
                    s_ps[:, nj, :],
                    lhsT=xsl(b, t, nj),
                    rhs=wT[:, 2 * t:2 * t + 2, :],
                    start=fresh[0], stop=(t == 1), perf_mode=DR,
                    skip_group_check=True,
                )
                fresh[0] = False
        return s_ps

    def gen_mms(b, J, P):
        """x^T gen for double-chunk J, cc-pair P. Returns PSUM tile."""
        g = ps_g.tile([128, 2, 256], F32, tag="g", name=f"g{b}_{J}_{P}")
        for tp in range(2):
            nc.tensor.matmul(
                g[:, tp, :],
                lhsT=xsl(b, P, 2 * J + tp),
                rhs=ibig,
                start=True, stop=True, perf_mode=DR,
            )
        return g

    def gen_drain(b, J, P, g, eng):
        st = stpool.tile([128, 2, 2, 128], FP8, tag=f"S{J}{P}",
                         name=f"S{b}_{J}_{P}", bufs=NGEN + 1)
        if eng is nc.scalar:
            nc.scalar.copy(st, g)
        else:
            nc.vector.tensor_copy(st, g)
        stages[(b, J, P)] = st

    def gen_J(b, J, e0, e1):
        gen_drain(b, J, 0, gen_mms(b, J, 0), e0)
        gen_drain(b, J, 1, gen_mms(b, J, 1), e1)

    def exp_op(b, njs):
        s_ps = s_tiles[b][0]
        if b not in Es:
            Es[b] = epool.tile([128, NB, K], BF16, tag="E", name=f"E{b}")
        E = Es[b]
        nj0, nj1 = njs[0], njs[-1] + 1
        nc.scalar.activation(E[:, nj0:nj1, :], s_ps[:, nj0:nj1, :],
                             func=AF.Exp)
        return E

    def softmax_tail(b, njs, mul_eng):
        E = Es[b]
        if b not in reds:
            reds[b] = spool.tile([128, NB], F32, tag="red", name=f"red{b}")
            recs[b] = spool.tile([128, NB], F32, tag="rec", name=f"rec{b}")
            ANs[b] = apool.tile([128, NB, K], FP8, tag="AN", name=f"AN{b}")
        red, rec, AN = reds[b], recs[b], ANs[b]
        nj0, nj1 = njs[0], njs[-1] + 1
        nc.vector.tensor_reduce(
            red[:, nj0:nj1], E[:, nj0:nj1, :],
            axis=mybir.AxisListType.X, op=mybir.AluOpType.add,
        )
        nc.vector.reciprocal(rec[:, nj0:nj1], red[:, nj0:nj1])
        rec_stride = rec.ap[-1][0]
        rec_b = bass.AP(
            tensor=rec.tensor,
            offset=rec.offset + nj0 * rec_stride,
            ap=[rec.ap[0], [rec_stride, nj1 - nj0], [0, K]],
        )
        mul_eng.tensor_mul(AN[:, nj0:nj1, :], E[:, nj0:nj1, :], rec_b)

    def v_views(b):
        vb = v_tiles[b]
        return (vb[:, 0:256].rearrange("p (cc k) -> p cc k", cc=DC),
                vb[0:64, 256:258], vb[0:1, 320:384], vb[:, 384:448])

    def vlad_mms(b, J, first, last):
        if b not in v_tiles:
            # b2/b3 scavenge the scores ring: those banks are dead after
            # the last Exp, while the v ring is still held by b0/b1
            # until their final Vn multiplies drain.
            pool = ps_s if b >= 2 else ps_v
            tag = "s" if b >= 2 else "v"
            v_tiles[b] = pool.tile([128, 512], F32, tag=tag, name=f"v{b}")
        v_ps, as_ps, _, _ = v_views(b)
        rhs = ANs[b][:, 2 * J:2 * J + 2, :]
        for cc in range(DC):
            nc.tensor.matmul(
                v_ps[:, cc, :],
                lhsT=stages[(b, J, cc // 2)][:, :, cc % 2, :],
                rhs=rhs,
                start=(first and cc == 0), stop=False, perf_mode=DR,
                skip_group_check=True,
            )
        nc.tensor.matmul(
            as_ps, lhsT=rhs, rhs=ones2,
            start=False, stop=last, perf_mode=DR, skip_group_check=True,
        )

    def centers_mms(b):
        v_ps, as_ps, _, _ = v_views(b)
        diag = spool.tile([64, 64], BF16, tag="diag", name=f"diag{b}")
        nc.vector.tensor_scalar_mul(diag, id64, as_ps[:, 0:1])
        for cc in range(DC):
            nc.tensor.matmul(
                v_ps[:, cc, :], lhsT=cT[:, cc, :], rhs=diag,
                start=False, stop=True, skip_group_check=True,
            )

    def tail_a(b):
        """Square-drain V (ACT), column-sum ssq row (PE), reciprocal (DVE)."""
        v_ps, _, ssq, _ = v_views(b)
        vsq = vpool.tile([128, DC, K], BF16, tag="vsq", name=f"vsq{b}")
        nc.scalar.activation(vsq, v_ps, func=AF.Square)
        vsqs[b] = vsq
        for cc in range(DC):
            nc.tensor.matmul(
                ssq, lhsT=ones_col, rhs=vsq[:, cc, :],
                start=False, stop=(cc == DC - 1), skip_group_check=True,
            )
        nc.vector.reciprocal(q_all[:, b, :], ssq)

    def tail_sqrt(bs):
        b0, b1 = bs[0], bs[-1] + 1
        nc.scalar.activation(rinv_all[:, b0:b1, :], q_all[:, b0:b1, :],
                             func=AF.Sqrt, scale=1.0 / 64.0)

    def tail_b(b, cp_eng, split_out=False):
        v_ps, _, _, rb_ps = v_views(b)
        nc.tensor.matmul(rb_ps, lhsT=ones_row, rhs=rinv_all[:, b, :],
                         start=False, stop=True, skip_group_check=True)
        rb = spool.tile([128, K], F32, tag="rb", name=f"rbs{b}")
        if cp_eng is nc.scalar:
            nc.scalar.copy(rb, rb_ps)
        else:
            nc.vector.tensor_copy(rb, rb_ps)
        rb_h = bass.AP(tensor=rb.tensor, offset=rb.offset,
                       ap=[rb.ap[0], [0, DC // 2], [1, K]])
        vn = opool.tile([128, DC, K], F32, tag="vn", name=f"vn{b}")
        if split_out:
            for h in range(2):
                nc.vector.tensor_mul(
                    vn[:, 2 * h:2 * h + 2, :],
                    v_ps[:, 2 * h:2 * h + 2, :], rb_h)
                nc.sync.dma_start(desc_v[:, b, 2 * h:2 * h + 2, :],
                                  vn[:, 2 * h:2 * h + 2, :])
        else:
            for h in range(2):
                nc.vector.tensor_mul(
                    vn[:, 2 * h:2 * h + 2, :],
                    v_ps[:, 2 * h:2 * h + 2, :], rb_h)
            nc.sync.dma_start(desc_v[:, b, :, :], vn)

    # ================= emission schedule ==============================
    # --- b0 (quarters) ---
    scores_mms(0, range(0, 4))
    gen_J(0, 0, nc.scalar, nc.vector)
    gen_J(0, 1, nc.scalar, nc.vector)
    scores_mms(0, range(4, 8))
    exp_op(0, range(0, 8))
    gen_J(0, 2, nc.scalar, nc.vector)
    gen_J(0, 3, nc.scalar, nc.vector)
    softmax_tail(0, range(0, 8), nc.gpsimd)

    # --- b1 ---
    scores_mms(1, range(0, 8))
    exp_op(1, range(0, 8))
    for J in range(NJ):
        vlad_mms(0, J, first=(J == 0), last=(J == NJ - 1))
    gen_J(1, 0, nc.scalar, nc.vector)
    gen_J(1, 1, nc.scalar, nc.vector)
    softmax_tail(1, range(0, 8), nc.gpsimd)
    centers_mms(0)
    gen_J(1, 2, nc.scalar, nc.vector)
    gen_J(1, 3, nc.scalar, nc.vector)
    tail_a(0)

    # --- b2 (fully shipped x^T; tail runs last) ---
    scores_mms(2, range(0, 8))
    for J in range(NJ):
        vlad_mms(1, J, first=(J == 0), last=(J == NJ - 1))
    exp_op(2, range(0, 8))
    centers_mms(1)
    tail_a(1)

    # --- b3 (quarters; chase; fully shipped x^T) ---
    scores_mms(3, range(0, 2))
    scores_mms(3, range(2, 4))
    exp_op(3, range(0, 4))
    softmax_tail(3, range(0, 4), nc.gpsimd)
    scores_mms(3, range(4, 6))
    scores_mms(3, range(6, 8))
    exp_op(3, range(4, 8))
    softmax_tail(3, range(4, 8), nc.gpsimd)
    # b2's softmax reduce AFTER b3's chain so it doesn't block the DVE queue
    softmax_tail(2, range(0, 8), nc.gpsimd)

    # sqrt-table phase: the load starts right after the last Exp above
    tail_sqrt(range(0, 2))

    # b2 then b3 vlads go ahead of the b0/b1 broadcasts on the PE queue
    for J in range(NJ):
        vlad_mms(2, J, first=(J == 0), last=(J == NJ - 1))
    centers_mms(2)
    for J in range(NJ):
        vlad_mms(3, J, first=(J == 0), last=(J == NJ - 1))
    centers_mms(3)
    tail_a(2)
    tail_sqrt(range(2, 3))
    tail_b(0, nc.vector)
    tail_b(1, nc.vector)
    tail_a(3)
    tail_sqrt(range(3, 4))
    tail_b(2, nc.scalar)
    tail_b(3, nc.scalar)


_NC_CACHE = None


def _build_nc():
    global _NC_CACHE
    if _NC_CACHE is not None:
        return _NC_CACHE
    from contextlib import ExitStack

    nc = bacc.Bacc("TRN2", target_bir_lowering=False, debug=False,
                   num_devices=NCORES)
    xn = nc.dram_tensor("xn", [BC, 4, 128, DC, N // 4], FP8,
                        kind="ExternalInput").ap()
    xt = nc.dram_tensor("xt", [BC - NGEN, NJ, 128, 2, 2, 2, 128], FP8,
                        kind="ExternalInput").ap()
    w = nc.dram_tensor("wt", [128, DC, K], FP8, kind="ExternalInput").ap()
    c = nc.dram_tensor("ct", [K, DC, 128], BF16, kind="ExternalInput").ap()
    out = nc.dram_tensor("desc", [BC, D * K], F32, kind="ExternalOutput").ap()
    with tile.TileContext(nc) as tc, ExitStack() as ctx:
        _netvlad_core(ctx, tc, out, xn, xt, w, c)
    nc.compile()
    _NC_CACHE = nc
    return nc


def _prep_inputs(x, conv_w, centers):
    """Host-side sharding + layout prep (fp8/bf16 cast, staging)."""
    wt = np.ascontiguousarray(
        conv_w.T.reshape(128, DC, K)
    ).astype(NP_FP8)
    ct = np.ascontiguousarray(
        (-centers.reshape(128, DC, K)).transpose(2, 1, 0)
    ).astype(NP_BF16)
    in_maps = []
    for i in range(NCORES):
        xc = x[i * BC:(i + 1) * BC]
        xn = np.ascontiguousarray(
            xc.reshape(BC, 128, DC, 4, N // 4).transpose(0, 3, 1, 2, 4)
        ).astype(NP_FP8)
        # xt[s, J, np, tp, P, u, i] = x[NGEN+s, 4i+2P+u, 256J+128tp+np]
        xs = xc[NGEN:].reshape(BC - NGEN, 128, 2, 2, NJ, 2, 128)
        #      [s, i, P, u, J, tp, np]
        xt = np.ascontiguousarray(
            xs.transpose(0, 4, 6, 5, 2, 3, 1)
        ).astype(NP_FP8)
        in_maps.append({"xn": xn, "xt": xt, "wt": wt, "ct": ct})
    return in_maps


def kernel(x, conv_w, centers):
    x = np.ascontiguousarray(x, dtype=np.float32)
    conv_w = np.ascontiguousarray(conv_w, dtype=np.float32)
    centers = np.ascontiguousarray(centers, dtype=np.float32)
    nc = _build_nc()
    in_maps = _prep_inputs(x, conv_w, centers)
    res = run_bass_kernel_spmd(nc, in_maps, core_ids=list(range(NCORES)))
    return np.concatenate([r["desc"] for r in res.results], axis=0)
